# revision 1
# baseline (speedup 1.0000x reference)
"""Trainium2 Bass/Tile kernel for nn_Capsule_6004364280312.

Computes CapsNet dynamic routing:
    u_hat = einsum('bnd,dm->bnm', u_vecs, W[0]) reshaped to [B, NC, N, DC]
    3 rounds of routing (softmax over N / weighted sum / squash / agreement)
    returns v [B, NC, DC]

Strategy (per core, batch-parallel over 8 cores, 4 batches each):
  * never materialize u_hat (268 MB). Algebra:
        s[i]  = (e[i] @ u) @ W_i          (e = exp(b), unnormalized softmax)
        b[i] += u @ (W_i @ (s[i] * rsqrt(||s[i]||^2 + eps)))
    The softmax normalizer cancels: v = normalize(s) is invariant to row
    scaling of e, so softmax is just exp().
  * partition layout p = bl*32 + i  (bl = local batch 0..3, i = capsule 0..31)
    so per-round tensors are full-width [128, *].
  * all matmul operands in float32r (TF32-like, 4x faster PE than fp32;
    measured rel err ~1.5e-4 per matmul, final ~3e-4, resid_var ~1e-7).
    f32r matmuls require dst partition base 0, so the per-batch (cu/bu)
    contractions run over the concatenated contraction axis with
    block-masked weights.
  * block-diagonal extraction of s from the full [128, 2048] product via a
    DRAM bounce with strided (diagonal) access patterns.
  * scatter/masked writes are single strided-AP ops; DMA count is minimized
    (the DMA queue engine costs ~620ns per dma_start).
"""

import numpy as np
from contextlib import ExitStack

import concourse.bass as bass
import concourse.mybir as mybir
import concourse.tile as tile
from concourse import bacc, bass_utils
from concourse.masks import make_identity

F32 = mybir.dt.float32
F32R = mybir.dt.float32r
BF16 = mybir.dt.bfloat16
AF = mybir.ActivationFunctionType
ALU = mybir.AluOpType

B, N, D = 32, 1024, 256
NC, DC = 32, 64
M = NC * DC  # 2048
N_CORES = 8
BL = B // N_CORES  # local batches per core
P = 128
EPS = 1e-7
ROUTINGS = 3


def _ap(base, offset, dims):
    """Raw strided AP over the same tensor as `base` (flat element space)."""
    return bass.AP(tensor=base.tensor, offset=offset, ap=dims)


def _build_kernel():
    nc = bacc.Bacc("TRN2", target_bir_lowering=False, debug=False,
                   num_devices=N_CORES)
    u_d = nc.dram_tensor("u", (BL * N, D), F32, kind="ExternalInput").ap()
    w_d = nc.dram_tensor("w", (D, M), F32, kind="ExternalInput").ap()
    v_d = nc.dram_tensor("v", (P, DC), F32, kind="ExternalOutput").ap()
    sf_d = nc.dram_tensor("sf_scratch", (P, M), F32, kind="Internal").ap()
    sfb_d = nc.dram_tensor("sfb_scratch", (P, M), BF16, kind="Internal").ap()

    with tile.TileContext(nc) as tc:
        with ExitStack() as ctx:
            _body(ctx, tc, v_d, u_d, w_d, sf_d, sfb_d)
    nc.compile()
    return nc


def _body(ctx, tc, v_d, u_d, w_d, sf_d, sfb_d):
    nc = tc.nc
    const = ctx.enter_context(tc.tile_pool(name="const", bufs=1))
    work = ctx.enter_context(tc.tile_pool(name="work", bufs=2))
    stage = ctx.enter_context(tc.tile_pool(name="stage", bufs=2))
    bstage = ctx.enter_context(tc.tile_pool(name="bstage", bufs=4))
    pquad = ctx.enter_context(tc.tile_pool(name="pquad", bufs=2, space="PSUM"))
    pmm = ctx.enter_context(tc.tile_pool(name="pmm", bufs=2, space="PSUM"))
    pbig = ctx.enter_context(tc.tile_pool(name="pbig", bufs=1, space="PSUM"))

    # ---------------- persistent SBUF state ----------------
    ident = const.tile([P, P], F32)
    make_identity(nc, ident)
    ident_r = const.tile([P, P], F32R)
    nc.gpsimd.tensor_copy(out=ident_r[:], in_=ident[:])
    ident_b = const.tile([P, P], BF16)
    nc.gpsimd.tensor_copy(out=ident_b[:], in_=ident[:])
    eps_sb = const.tile([P, 1], F32)
    nc.gpsimd.memset(eps_sb[:].bitcast(F32), EPS)
    EPS_SB_BIAS = eps_sb[:]

    # block-masked all-ones weights for round 0 (uniform softmax):
    # onesm[bl] = [128, 128] with cols [32bl, 32bl+32) = 1, else 0
    onesm = const.tile([P, BL * P], F32R)
    nc.gpsimd.memset(onesm[:].bitcast(F32), 0.0)
    nc.gpsimd.memset(
        _ap(onesm[:], 0, [[BL * P, P], [P + 32, BL], [1, 32]]).bitcast(F32), 1.0)

    u_sb = const.tile([P, BL * 8 * D], F32R)   # u[bl][jk]: [128(j), 256(d)]
    uT_sb = const.tile([P, BL * 2 * N], BF16)  # uT[bl][dk]: [128(d), 1024(j)]
    w_sb = const.tile([P, 2 * M], F32R)        # w[dk]: [128(d), 2048(m)]
    wT_sb = const.tile([P, 16 * D], BF16)      # wT[mk]: [128(m), 256(d)]
    bT_sb = const.tile([P, N], F32)            # routing logits, [j, p] layout
    # block-masked exp(b)^T: eTm[(bl,jk)][j_local, p] = e[p, jk*128+j_local]
    # for p in bl's block, else 0
    eTm = const.tile([P, BL * 8 * P], F32R)
    nc.gpsimd.memset(eTm[:].bitcast(F32), 0.0)
    # block-masked wv^T: wvm[(bl,dk)][d_local, p] masked to bl's block
    wvm = const.tile([P, BL * 2 * P], BF16)
    nc.gpsimd.memset(wvm[:], 0.0)
    vemb = const.tile([P, 16 * P], BF16)       # block-diag s embedding
    nc.gpsimd.memset(vemb[:], 0.0)

    copy_engines = [nc.scalar.copy, nc.vector.tensor_copy]

    # ------- loads: W and u land in f32r (v-path) + bf16 (transpose) stages
    wbf = []
    for dk in range(2):
        wst = stage.tile([P, M], F32, tag="wst")
        dma_w = nc.sync.dma_start if dk == 0 else nc.scalar.dma_start
        dma_w(out=wst[:], in_=w_d[dk * 128:(dk + 1) * 128, :])
        for half in range(2):
            copy_engines[half](
                out=w_sb[:, dk * M + half * 1024: dk * M + (half + 1) * 1024],
                in_=wst[:, half * 1024:(half + 1) * 1024])
        wb = bstage.tile([P, M], BF16, tag="wbf")
        copy_engines[dk](out=wb[:], in_=wst[:])
        wbf.append(wb)
    ubf = []
    for bl in range(BL):
        ust = stage.tile([P, 8 * D], F32, tag="ust")
        # gather the 8 j-tiles of batch bl in one DMA:
        # dst[p, (jk, d)] = u[bl*1024 + jk*128 + p, d]
        srcu = _ap(u_d, bl * N * D, [[D, P], [P * D, 8], [1, D]])
        dma_u = [nc.sync.dma_start, nc.gpsimd.dma_start,
                 nc.scalar.dma_start, nc.gpsimd.dma_start][bl]
        dma_u(out=ust[:].rearrange("p (jk d) -> p jk d", jk=8), in_=srcu)
        for half in range(2):
            copy_engines[half](
                out=u_sb[:, bl * 8 * D + half * 1024:
                         bl * 8 * D + (half + 1) * 1024],
                in_=ust[:, half * 1024:(half + 1) * 1024])
        ub = bstage.tile([P, 8 * D], BF16, tag="ubf")
        copy_engines[bl % 2](out=ub[:], in_=ust[:])
        ubf.append(ub)

    def emit_transposes():
        # W^T: for fixed dk the 16 mk-blocks are stride-256 in wT
        for dk in range(2):
            for g in range(4):
                pt = pquad.tile([P, 4 * P], BF16, tag="quad")
                for q in range(4):
                    mk = g * 4 + q
                    nc.tensor.transpose(out=pt[:, q * P:(q + 1) * P],
                                        in_=wbf[dk][:, mk * 128:(mk + 1) * 128],
                                        identity=ident_b[:])
                dst = _ap(wT_sb[:], (g * 4) * D + dk * 128,
                          [[16 * D, P], [D, 4], [1, P]])
                copy_engines[(dk * 4 + g) % 2](
                    out=dst, in_=pt[:].rearrange("p (q c) -> p q c", q=4))
        # u^T
        for bl in range(BL):
            for dk in range(2):
                for g in range(2):
                    pt = pquad.tile([P, 4 * P], BF16, tag="quad")
                    for q in range(4):
                        jk = g * 4 + q
                        nc.tensor.transpose(
                            out=pt[:, q * P:(q + 1) * P],
                            in_=ubf[bl][:, jk * D + dk * 128:
                                        jk * D + (dk + 1) * 128],
                            identity=ident_b[:])
                    copy_engines[(bl * 4 + dk * 2 + g) % 2](
                        out=uT_sb[:, (bl * 2 + dk) * N + g * 512:
                                  (bl * 2 + dk) * N + (g + 1) * 512],
                        in_=pt[:])

    # ---------------- routing rounds ----------------
    for r in range(ROUTINGS):
        last_round = (r == ROUTINGS - 1)
        # cu[p, d] = sum_j e[p, j] * u[bl(p)][j, d] as one accumulation over
        # the concatenated (bl, jk) axis with block-masked weights
        cu_ps = pmm.tile([P, D], F32, tag="mm")
        first, last = (0, 0), (BL - 1, 7)
        for bl in range(BL):
            for jk in range(8):
                lhs = (onesm[:, bl * P:(bl + 1) * P] if r == 0 else
                       eTm[:, (bl * 8 + jk) * P:(bl * 8 + jk + 1) * P])
                nc.tensor.matmul(
                    out=cu_ps[:],
                    lhsT=lhs,
                    rhs=u_sb[:, (bl * 8 + jk) * D:(bl * 8 + jk + 1) * D],
                    start=((bl, jk) == first), stop=((bl, jk) == last))
        cu_sb = work.tile([P, D], F32R, tag="cu")
        nc.vector.tensor_copy(out=cu_sb[:], in_=cu_ps[:])
        cuT_sb = work.tile([P, D], F32R, tag="cuT")
        pt = pquad.tile([P, 2 * P], F32R, tag="quad")
        for dk in range(2):
            nc.tensor.transpose(out=pt[:, dk * P:(dk + 1) * P],
                                in_=cu_sb[:, dk * 128:(dk + 1) * 128],
                                identity=ident_r[:])
        nc.vector.tensor_copy(out=cuT_sb[:], in_=pt[:])

        # s_full[p, m] = sum_d cu[p, d] * W[d, m], then DRAM-bounce to
        # extract diagonal blocks: s[p, d'] = s_full[p, i(p)*64 + d'].
        # Rounds 0/1 only feed the agreement path -> bf16 bounce; the last
        # round's s becomes v -> f32 bounce.
        sdt = F32 if last_round else BF16
        sdram = sf_d if last_round else sfb_d
        sf_ps = pbig.tile([P, M], F32, tag="big")
        for n in range(4):
            for dk in range(2):
                nc.tensor.matmul(
                    out=sf_ps[:, n * 512:(n + 1) * 512],
                    lhsT=cuT_sb[:, dk * 128:(dk + 1) * 128],
                    rhs=w_sb[:, dk * M + n * 512: dk * M + (n + 1) * 512],
                    start=(dk == 0), stop=(dk == 1))
        sf_sb = work.tile([P, M], sdt, tag="sf")
        for q in range(2):
            copy_engines[q](out=sf_sb[:, q * 1024:(q + 1) * 1024],
                            in_=sf_ps[:, q * 1024:(q + 1) * 1024])
        nc.sync.dma_start(out=sdram[:, 0:1024], in_=sf_sb[:, 0:1024])
        nc.gpsimd.dma_start(out=sdram[:, 1024:2048], in_=sf_sb[:, 1024:2048])
        s_sb = work.tile([P, DC], sdt, tag="s")
        for bl in range(BL):
            srcd = _ap(sdram, bl * 32 * M, [[M + DC, 32], [1, DC]])
            nc.scalar.dma_start(out=s_sb[bl * 32:(bl + 1) * 32, :], in_=srcd)

        if r == 0:
            emit_transposes()

        # squash scale: rv = 1/sqrt(sum(s^2) + eps), entirely on DVE
        # (bit-trick seed + 3 Newton iterations; keeps ACT's table on Exp)
        sq_sb = work.tile([P, DC], F32, tag="sq")
        ssq = work.tile([P, 1], F32, tag="ssq")
        nc.vector.scalar_tensor_tensor(out=sq_sb[:], in0=s_sb[:], scalar=1.0,
                                       in1=s_sb[:], op0=ALU.mult,
                                       op1=ALU.mult, accum_out=ssq[:])
        sr = work.tile([P, 1], F32, tag="sr")
        nc.scalar.activation(out=sr[:], in_=ssq[:], func=AF.Sqrt, bias=EPS_SB_BIAS)
        rv = work.tile([P, 1], F32, tag="rv")
        nc.vector.reciprocal(out=rv[:], in_=sr[:])

        if last_round:
            v_sb = work.tile([P, DC], F32, tag="v")
            nc.vector.tensor_scalar(out=v_sb[:], in0=s_sb[:],
                                    scalar1=rv[:, 0:1], scalar2=None,
                                    op0=ALU.mult)
            nc.sync.dma_start(out=v_d[:], in_=v_sb[:])
            continue

        # s2 = [s, s] duplicated along free dim; s2T[t*64+d', p] = s[p, d']
        s2_sb = work.tile([P, 2 * DC], BF16, tag="s2")
        nc.scalar.copy(out=s2_sb[:].rearrange("p (t c) -> p t c", t=2),
                       in_=s_sb[:].unsqueeze(1).to_broadcast([P, 2, DC]))
        pt2 = pquad.tile([P, P], BF16, tag="quad")
        nc.tensor.transpose(out=pt2[:], in_=s2_sb[:], identity=ident_b[:])
        # scatter s into the block-diagonal embedding vemb (from psum):
        # vemb_k[t*64+d', p] = s[p, d'] for p with capsule i(p) == 2k+t
        for t in range(2):
            srcv = _ap(pt2[:], t * 64 * P + t, [[P, 64], [2, 16], [32, 4]])
            dstv = _ap(vemb[:], t * 64 * (16 * P) + t,
                       [[16 * P, 64], [P + 2, 16], [32, 4]])
            copy_engines[t](out=dstv, in_=srcv)

        # w_v[p, d] = sum_{d'} s[p, d'] * W[d, i(p)*64+d']
        wv_ps = pmm.tile([P, D], F32, tag="mm")
        for k in range(16):
            nc.tensor.matmul(out=wv_ps[:],
                             lhsT=vemb[:, k * P:(k + 1) * P],
                             rhs=wT_sb[:, k * D:(k + 1) * D],
                             start=(k == 0), stop=(k == 15))
        # scale by rv while copying out of psum
        wv_sb = work.tile([P, D], BF16, tag="wv")
        nc.vector.tensor_scalar(out=wv_sb[:], in0=wv_ps[:],
                                scalar1=rv[:, 0:1], scalar2=None, op0=ALU.mult)
        # transpose wv (both halves into one psum quad), then one 4-level-AP
        # copy scatters both dk blocks into the masked wvm tiles
        ptw = pquad.tile([P, 2 * P], BF16, tag="quad")
        for dk in range(2):
            nc.tensor.transpose(out=ptw[:, dk * P:(dk + 1) * P],
                                in_=wv_sb[:, dk * 128:(dk + 1) * 128],
                                identity=ident_b[:])
        dstw = _ap(wvm[:], 0,
                   [[BL * 2 * P, P], [2 * P + 32, BL], [P, 2], [1, 32]])
        srcw = _ap(ptw[:], 0, [[2 * P, P], [32, BL], [P, 2], [1, 32]])
        nc.vector.tensor_copy(out=dstw, in_=srcw)

        # bu^T[j, p] = sum_d uT[bl(p)][d, j] * wvm[d, p]  (transposed output:
        # keeps b in [j, p] layout so no per-round b transposes are needed)
        buT_ps = pbig.tile([P, N], F32, tag="big")
        for jc in range(8):
            for bl in range(BL):
                for dk in range(2):
                    nc.tensor.matmul(
                        out=buT_ps[:, jc * 128:(jc + 1) * 128],
                        lhsT=uT_sb[:, (bl * 2 + dk) * N + jc * 128:
                                   (bl * 2 + dk) * N + (jc + 1) * 128],
                        rhs=wvm[:, (bl * 2 + dk) * P:(bl * 2 + dk + 1) * P],
                        start=(bl == 0 and dk == 0),
                        stop=(bl == 3 and dk == 1))

        # bT += buT ; eTm = masked exp(bT) — one strided activation for all
        for g in range(2):
            gsl = slice(g * 512, (g + 1) * 512)
            if r == 0:
                nc.vector.tensor_copy(out=bT_sb[:, gsl], in_=buT_ps[:, gsl])
            else:
                nc.vector.tensor_add(out=bT_sb[:, gsl], in0=bT_sb[:, gsl],
                                     in1=buT_ps[:, gsl])
            # eTm col for (bl, jk, c) = bl*1056 + jk*128 + c;
            # bT col for (bl, jk, c) = jk*128 + bl*32 + c
            dste = _ap(eTm[:], g * 512,
                       [[BL * 8 * P, P], [8 * P + 32, BL], [P, 4], [1, 32]])
            srce = _ap(bT_sb[:], g * 512,
                       [[N, P], [32, BL], [P, 4], [1, 32]])
            nc.scalar.activation(out=dste, in_=srce, func=AF.Exp)


_NC_CACHE = None


def _get_nc():
    global _NC_CACHE
    if _NC_CACHE is None:
        _NC_CACHE = _build_kernel()
    return _NC_CACHE


def kernel(u_vecs: np.ndarray, W: np.ndarray) -> np.ndarray:
    u_vecs = np.ascontiguousarray(np.asarray(u_vecs, dtype=np.float32))
    W0 = np.ascontiguousarray(np.asarray(W, dtype=np.float32).reshape(D, M))
    nc = _get_nc()
    in_maps = [
        {"u": u_vecs[c * BL:(c + 1) * BL].reshape(BL * N, D), "w": W0}
        for c in range(N_CORES)
    ]
    res = bass_utils.run_bass_kernel_spmd(nc, in_maps,
                                          core_ids=list(range(N_CORES)))
    out = np.empty((B, NC, DC), dtype=np.float32)
    for c in range(N_CORES):
        out[c * BL:(c + 1) * BL] = res.results[c]["v"].reshape(BL, NC, DC)
    return out



# revision 15
# speedup vs baseline: 1.9366x; 1.9366x over previous
"""Trainium2 Bass/Tile kernel for nn_Capsule_6004364280312.

CapsNet dynamic routing, batch-parallel over 8 cores (4 batches/core).

Transposed-space formulation. Per core, 128 (batch, capsule) pairs live on
the free axis of every routing tensor ("p-columns"); two p-orders are used
so every matmul OUTPUT is a contiguous psum slice (only rhs reads are
strided):
  * bT/eT/cu space: p = bl*32 + i      (contiguous per-bl 32-col blocks)
  * sT/wv/rv space: p = i*4 + bl       (contiguous per-capsule 4-col blocks)

Algebra (e = exp(b); the softmax normalizer cancels under the final L2
normalization):
  cuT[d, p]  = sum_j u[bl(p)][j, d] * e[p, j]     64 matmuls x 32 cols
  sT[d', p]  = sum_d W[d, i(p)*64+d'] * cuT[d, p] 64 matmuls x 4 cols
               (per-capsule diagonal W block -> no [128,2048] product,
                no DRAM bounce)
  rv[p]      = 1/sqrt(colsum sT^2 + eps)          ones-matmul colsum +
               ACT ln/exp (single act table, loaded once, never switched)
  wvT[d, p]  = rv[p] * sum_d' W[d, i*64+d'] * sT[d', p]
               (rank-1 ones x rv matmul broadcast + DVE multiply)
  bT[j, p]  += sum_d uT[bl(p)][d, j] * wvT[d, p]  accumulated in PSUM
               across rounds (open accumulation group); eT = exp(bT)
  v[p, d']   = transpose(sT)[p, d'] * rv[p]       (last round only)

Cost-model notes: matmul time = out-free-cols x pe_cycle x rate(moving
dtype); every moving operand here is bf16 (rate 1.0) except two tiny rv
broadcasts. Inputs land in SBUF as f32 via 6 x 1MB DMAs (the HBM roofline,
~17.5us) and are bitcast to f32r for stationary use; u/W transposes run on
PE inside the DMA window. Round 0 exploits uniform softmax: cu = colsum(u)
via ones-rhs matmuls (64 x 1 col).
"""

import numpy as np
from contextlib import ExitStack

import concourse.bass as bass
import concourse.mybir as mybir
import concourse.tile as tile
from concourse import bacc, bass_utils
from concourse.masks import make_identity

F32 = mybir.dt.float32
F32R = mybir.dt.float32r
BF16 = mybir.dt.bfloat16
AF = mybir.ActivationFunctionType
ALU = mybir.AluOpType

B, N, D = 32, 1024, 256
NC, DC = 32, 64
M = NC * DC  # 2048
N_CORES = 8
BL = B // N_CORES  # 4 local batches per core
P = 128
EPS = 1e-7
ROUTINGS = 3


def _ap(base, offset, dims):
    """Raw strided AP over the same tensor as `base` (flat element space,
    offset relative to the tile's backing tensor)."""
    return bass.AP(tensor=base.tensor, offset=offset, ap=dims)


def _build_kernel():
    nc = bacc.Bacc("TRN2", target_bir_lowering=False, debug=False,
                   num_devices=N_CORES)
    u_d = nc.dram_tensor("u", (BL * N, D), F32, kind="ExternalInput").ap()
    w_d = nc.dram_tensor("w", (D, M), F32, kind="ExternalInput").ap()
    v_d = nc.dram_tensor("v", (P, DC), F32, kind="ExternalOutput").ap()

    with tile.TileContext(nc) as tc:
        with ExitStack() as ctx:
            _body(ctx, tc, v_d, u_d, w_d)
    nc.compile()
    return nc


def _body(ctx, tc, v_d, u_d, w_d):
    nc = tc.nc
    const = ctx.enter_context(tc.tile_pool(name="const", bufs=1))
    work = ctx.enter_context(tc.tile_pool(name="work", bufs=2))
    pbig = ctx.enter_context(tc.tile_pool(name="pbig", bufs=1, space="PSUM"))
    pquad = ctx.enter_context(tc.tile_pool(name="pquad", bufs=2, space="PSUM"))
    pcw = ctx.enter_context(tc.tile_pool(name="pcw", bufs=2, space="PSUM"))
    psmall = ctx.enter_context(tc.tile_pool(name="psmall", bufs=1, space="PSUM"))

    copy_engines = [nc.scalar.copy, nc.vector.tensor_copy]

    # persistent PSUM: routing logits bT[j_local, jc*128 + bl*32 + i]
    pbT = pbig.tile([P, N], F32)

    # ---------------- constants ----------------
    ident_f = const.tile([P, P], F32)
    make_identity(nc, ident_f[:])
    ident_b = const.tile([P, P], BF16)
    nc.gpsimd.tensor_copy(out=ident_b[:], in_=ident_f[:])
    ident_r = const.tile([P, P], F32R)
    nc.gpsimd.tensor_copy(out=ident_r[:], in_=ident_f[:])
    ones_col = const.tile([P, 1], BF16)
    nc.gpsimd.memset(ones_col[:], 1.0)
    ones_row = const.tile([1, P], F32R)
    nc.gpsimd.memset(ones_row[:].bitcast(F32), 1.0)
    eps1 = const.tile([1, 1], F32)
    nc.gpsimd.memset(eps1[:], EPS)

    # ---------------- input DMA: u0, W0, W1, u1, u2, u3 ----------------
    u_sb = [const.tile([P, 8 * D], F32, name=f"u_sb{bl}", tag=f"u{bl}")
            for bl in range(BL)]
    w_sb = [const.tile([P, M], F32, name=f"w_sb{dk}", tag=f"w{dk}")
            for dk in range(2)]

    def load_u(bl):
        srcu = _ap(u_d, bl * N * D, [[D, P], [P * D, 8], [1, D]])
        nc.sync.dma_start(
            out=u_sb[bl][:].rearrange("p (jk d) -> p jk d", jk=8), in_=srcu)

    load_u(0)
    nc.sync.dma_start(out=w_sb[0][:], in_=w_d[0:P, :])
    nc.sync.dma_start(out=w_sb[1][:], in_=w_d[P:2 * P, :])
    for bl in range(1, BL):
        load_u(bl)

    # bf16 staging (walrus rejects mixed 32/16-bit matmul operands, so all
    # 32-col matmuls use bf16 x bf16; staged on otherwise-idle engines)
    ub = [const.tile([P, 8 * D], BF16, name=f"ub{bl}", tag=f"ub{bl}")
          for bl in range(BL)]
    wb = [const.tile([P, M], BF16, name=f"wb{dk}", tag=f"wb{dk}")
          for dk in range(2)]
    stage_engines = [nc.vector.tensor_copy, nc.scalar.copy,
                     nc.gpsimd.tensor_copy]

    def stage_u(bl):
        stage_engines[bl % 3](out=ub[bl][:], in_=u_sb[bl][:])

    def stage_w(dk):
        stage_engines[(dk + 1) % 3](out=wb[dk][:], in_=w_sb[dk][:])

    # --------- one-time transposes + round-0 column sums (DMA window) ----
    # uT[bl][dk]: [128 d, 1024 j] bf16, slab jc at cols jc*128
    uT = [[const.tile([P, N], BF16, name=f"uT{bl}{dk}", tag=f"uT{bl}{dk}")
           for dk in range(2)] for bl in range(BL)]
    # wT: [64 d', 8192] bf16; capsule i half dk at cols i*256 + dk*128
    wT = const.tile([64, NC * 2 * P], BF16, tag="wT")
    # round-0 cu: cu0[d, dk*4 + bl] = sum_j u[bl][j, dk*128 + d]
    pc0 = pcw.tile([P, 2 * P], F32, tag="cuwv")

    def emit_uT(bl):
        for dk in range(2):
            for g in range(2):
                pt = pquad.tile([P, 4 * P], BF16, tag="quad")
                for q in range(4):
                    jc = g * 4 + q
                    nc.tensor.transpose(
                        out=pt[:, q * P:(q + 1) * P],
                        in_=ub[bl][:, jc * D + dk * P:jc * D + dk * P + P],
                        identity=ident_b[:])
                copy_engines[(bl + dk + g) % 2](
                    out=uT[bl][dk][:, g * 512:(g + 1) * 512], in_=pt[:])

    def emit_cu0(bl):
        # single accumulation epoch for the whole bank: exactly one start
        # (first matmul) and one stop (last); first touch of each column
        # auto-zeroes via the bank's pending-zero marking.
        for dk in range(2):
            for jk in range(8):
                nc.tensor.matmul(
                    out=pc0[:, dk * 4 + bl:dk * 4 + bl + 1],
                    lhsT=ub[bl][:, jk * D + dk * P:jk * D + dk * P + P],
                    rhs=ones_col[:, 0:1],
                    start=(bl == 0 and dk == 0 and jk == 0),
                    stop=(bl == BL - 1 and dk == 1 and jk == 7))

    def emit_wT(dk):
        for g in range(8):
            pt = pquad.tile([P, 4 * P], BF16, tag="quad")
            for q in range(4):
                i = g * 4 + q
                nc.tensor.transpose(
                    out=pt[0:64, q * P:(q + 1) * P],
                    in_=wb[dk][:, i * DC:(i + 1) * DC],
                    identity=ident_b[:])
            dst = _ap(wT[:], (g * 4) * 2 * P + dk * P,
                      [[NC * 2 * P, 64], [2 * P, 4], [1, P]])
            copy_engines[(dk + g) % 2](
                out=dst, in_=pt[0:64, :].rearrange("p (q c) -> p q c", q=4))

    stage_u(0)
    emit_uT(0)
    emit_cu0(0)
    for dk in range(2):
        stage_w(dk)
        emit_wT(dk)
    for bl in range(1, BL):
        stage_u(bl)
        emit_uT(bl)
        emit_cu0(bl)

    cu0_sb = work.tile([P, 8], BF16, tag="cu0sb")
    nc.vector.tensor_copy(out=cu0_sb[:], in_=pc0[:, 0:8])

    # ---------------- routing rounds ----------------
    eT = None  # four [128, 256] bf16 chunks, (jc, bl*32+i) columns
    for r in range(ROUTINGS):
        last = (r == ROUTINGS - 1)

        # ---- cuT[d, p] (bT-space p) ----
        if r > 0:
            pc = pcw.tile([P, 2 * P], F32, tag="cuwv")
            for jk in range(8):
                for dk in range(2):
                    for bl in range(BL):
                        nc.tensor.matmul(
                            out=pc[:, dk * P + bl * 32:dk * P + bl * 32 + 32],
                            lhsT=ub[bl][:, jk * D + dk * P:jk * D + dk * P + P],
                            rhs=eT[jk // 2][:, (jk % 2) * P + bl * 32:
                                            (jk % 2) * P + bl * 32 + 32],
                            start=(jk == 0 and dk == 0 and bl == 0),
                            stop=(jk == 7 and dk == 1 and bl == BL - 1))
            cu_sb = work.tile([P, 2 * P], BF16, tag="cusb")
            nc.vector.tensor_copy(out=cu_sb[:], in_=pc[:])

        # ---- small psum pack: cols 0:128 sT (parts 0:64), 128:256 rvb,
        # 256:384 ssq row, 384:448 vt, 448:512 rvb2 ----
        ps = psmall.tile([P, 4 * P], F32, tag="ps")
        psT = ps[0:64, 0:P]
        for i in range(NC):
            for dk in range(2):
                if r == 0:
                    rhs = cu0_sb[:, dk * 4:dk * 4 + 4]
                else:
                    rhs = _ap(cu_sb[:], dk * P + i, [[2 * P, P], [32, 4]])
                nc.tensor.matmul(
                    out=psT[:, i * 4:(i + 1) * 4],
                    lhsT=wb[dk][:, i * DC:(i + 1) * DC],
                    rhs=rhs, start=(i == 0 and dk == 0),
                    stop=(i == NC - 1 and dk == 1))

        # ---- squash scale rv[p] = 1/sqrt(colsum sT^2 + eps) ----
        sq_sb = work.tile([64, P], BF16, tag="sq")
        nc.scalar.activation(out=sq_sb[:], in_=psT, func=AF.Square)
        pq = ps[0:1, 2 * P:3 * P]
        nc.tensor.matmul(out=pq, lhsT=ones_col[0:64, 0:1], rhs=sq_sb[:],
                         start=True, stop=True)
        t_sb = work.tile([1, P], F32, tag="lnt")
        nc.scalar.activation(out=t_sb[:], in_=pq, func=AF.Ln,
                             bias=eps1[:, 0:1])
        rv_sb = work.tile([1, P], F32R, tag="rv")
        nc.scalar.activation(out=rv_sb[:], in_=t_sb[:], func=AF.Exp,
                             scale=-0.5)

        if last:
            # ---- v[p, d'] = transpose(sT) * rv ----
            s_sb = work.tile([64, P], F32R, tag="slast")
            nc.vector.tensor_copy(out=s_sb[:], in_=psT)
            pvt = ps[:, 6 * DC:7 * DC]
            nc.tensor.transpose(out=pvt.bitcast(F32R),
                                in_=s_sb[:],
                                identity=ident_r[0:64, 0:64])
            prv2 = ps[:, 7 * DC:8 * DC]
            nc.tensor.matmul(out=prv2, lhsT=rv_sb[:],
                             rhs=ones_row[0:1, 0:DC],
                             start=True, stop=True)
            rv2_sb = work.tile([P, DC], F32, tag="rv2")
            nc.scalar.copy(out=rv2_sb[:], in_=prv2)
            v_sb = work.tile([P, DC], F32, tag="v")
            nc.vector.tensor_tensor(out=v_sb[:], in0=pvt, in1=rv2_sb[:],
                                    op=ALU.mult)
            # v rows land in sT-space order p' = i*4 + bl; the host wrapper
            # permutes back to (bl, i) for free.
            nc.sync.dma_start(out=v_d[:], in_=v_sb[:])
            continue

        # ---- wvT[d, dk*128 + i*4 + bl] = rv * (wT . sT) ----
        sTr = work.tile([64, P], BF16, tag="sTr")
        nc.vector.tensor_copy(out=sTr[:], in_=psT)
        pw = pcw.tile([P, 2 * P], F32, tag="cuwv")
        for i in range(NC):
            for dk in range(2):
                nc.tensor.matmul(
                    out=pw[:, dk * P + i * 4:dk * P + (i + 1) * 4],
                    lhsT=wT[0:64, i * 2 * P + dk * P:i * 2 * P + (dk + 1) * P],
                    rhs=sTr[:, i * 4:(i + 1) * 4],
                    start=True, stop=True)
        prv = ps[:, P:2 * P]
        nc.tensor.matmul(out=prv, lhsT=ones_row[0:1, :],
                         rhs=rv_sb[:], start=True, stop=True)
        rvb_sb = work.tile([P, P], F32, tag="rvb")
        nc.scalar.copy(out=rvb_sb[:], in_=prv)
        wv_sb = work.tile([P, 2 * P], BF16, tag="wvsb")
        for dk in range(2):
            nc.vector.tensor_tensor(out=wv_sb[:, dk * P:(dk + 1) * P],
                                    in0=pw[:, dk * P:(dk + 1) * P],
                                    in1=rvb_sb[:], op=ALU.mult)

        # ---- bT[j, jc*128 + bl*32 + i] += uT . wvT (PSUM accumulation
        # held open across rounds; reads between rounds are safe) ----
        for dk in range(2):
            for jc in range(8):
                for bl in range(BL):
                    nc.tensor.matmul(
                        out=pbT[:, jc * P + bl * 32:jc * P + bl * 32 + 32],
                        lhsT=uT[bl][dk][:, jc * P:(jc + 1) * P],
                        rhs=_ap(wv_sb[:], dk * P + bl, [[2 * P, P], [4, 32]]),
                        start=(r == 0 and dk == 0 and bl == 0
                               and jc in (0, 4)),
                        stop=(r == 1 and dk == 1 and bl == BL - 1
                              and jc in (3, 7)),
                        skip_group_check=True)

        # ---- eT = exp(bT), 4 chunks to pipeline with next round's cu ----
        eT = [work.tile([P, 2 * P], BF16, name=f"eT{c}", tag=f"eT{c}")
              for c in range(4)]
        for c in range(4):
            nc.scalar.activation(out=eT[c][:], in_=pbT[:, c * 2 * P:(c + 1) * 2 * P],
                                 func=AF.Exp)


_NC_CACHE = None


def _get_nc():
    global _NC_CACHE
    if _NC_CACHE is None:
        _NC_CACHE = _build_kernel()
    return _NC_CACHE


def kernel(u_vecs: np.ndarray, W: np.ndarray) -> np.ndarray:
    u_vecs = np.ascontiguousarray(np.asarray(u_vecs, dtype=np.float32))
    W0 = np.ascontiguousarray(np.asarray(W, dtype=np.float32).reshape(D, M))
    nc = _get_nc()
    in_maps = [
        {"u": u_vecs[c * BL:(c + 1) * BL].reshape(BL * N, D), "w": W0}
        for c in range(N_CORES)
    ]
    res = bass_utils.run_bass_kernel_spmd(nc, in_maps,
                                          core_ids=list(range(N_CORES)))
    out = np.empty((B, NC, DC), dtype=np.float32)
    for c in range(N_CORES):
        # device rows are p' = i*4 + bl -> [NC, BL] major order
        vc = res.results[c]["v"].reshape(NC, BL, DC).transpose(1, 0, 2)
        out[c * BL:(c + 1) * BL] = vc
    return out
